# revision 7
# baseline (speedup 1.0000x reference)
"""CrossAttentionBlock kernel for 8 Trainium2 NeuronCores.

Sharding: 16 (batch, head) pairs -> 8 cores, each core owns one batch b and
two heads (2*hp, 2*hp+1).  Per core:
  qT/kT = (Wq/Wk slice)^T-projection of condition[b]   [128=2*64 d, 4096 t]
  v     = x[b] @ Wv slice                               [4096 j, 128 dv]
  S^T   = kT^T-slices @ qT  (per head, row-packed on the PE)
  P     = exp(S^T - 8)  (ScalarE, PSUM->SBUF, bf16)
  out^T = v^T @ P^T  accumulated over j (col-packed 2 heads), Z via ones-matmul
  final = (out^T / Z)^T @ Wu slice  -> partial [4096, 512] fp32
Host sums the 4 per-batch partials and adds b_u.
"""

import numpy as np
import ml_dtypes

B, T, C = 2, 4096, 512
H, DH = 8, 64
COND = 512
SCALE = (DH // H) ** -0.5  # faithful to reference: 8**-0.5
NCORES = 8
DV = 2 * DH          # per-core head-pair width = 128
CK = COND // 128     # 4 contraction chunks
TJ = T // 128        # 32 key chunks
IB = 512             # query block
NIB = T // IB        # 8
EXP_BIAS = -8.0      # constant shift inside exp; cancels in the softmax ratio

_BUILT = None


def _build_nc():
    import concourse.bass as bass  # noqa: F401
    import concourse.tile as tile
    from concourse import bacc, mybir

    f32 = mybir.dt.float32
    bf16 = mybir.dt.bfloat16

    nc = bacc.Bacc(None)
    condT_d = nc.declare_dram_parameter("condT", [COND, T], bf16, isOutput=False)
    xT_d = nc.declare_dram_parameter("xT", [C, T], bf16, isOutput=False)
    Wq_d = nc.declare_dram_parameter("Wq", [COND, DV], bf16, isOutput=False)
    Wk_d = nc.declare_dram_parameter("Wk", [COND, DV], bf16, isOutput=False)
    Wv_d = nc.declare_dram_parameter("Wv", [C, DV], bf16, isOutput=False)
    Wu_d = nc.declare_dram_parameter("Wu", [DV, C], bf16, isOutput=False)
    out_d = nc.declare_dram_parameter("out", [T, C], f32, isOutput=True)

    Exp = mybir.ActivationFunctionType.Exp

    with tile.TileContext(nc) as tc:
        with (
            tc.tile_pool(name="persist", bufs=1) as persist,
            tc.tile_pool(name="work", bufs=3) as work,
            tc.tile_pool(name="outsb", bufs=3) as outsb,
            tc.tile_pool(name="pt_pool", bufs=4) as pt_pool,
            tc.tile_pool(name="stage_ps", bufs=2, space="PSUM") as stage_ps,
            tc.tile_pool(name="pv_ps", bufs=2, space="PSUM") as pv_ps,
            tc.tile_pool(name="zb_ps", bufs=2, space="PSUM") as zb_ps,
        ):
            # ---------------- load inputs ----------------
            condT_sb = persist.tile([128, CK, T], bf16)
            nc.sync.dma_start(
                condT_sb, condT_d.rearrange("(co ci) t -> ci co t", ci=128)
            )
            xT_sb = persist.tile([128, CK, T], bf16)
            nc.sync.dma_start(xT_sb, xT_d.rearrange("(co ci) t -> ci co t", ci=128))
            Wq_sb = persist.tile([128, CK, DV], bf16)
            nc.sync.dma_start(Wq_sb, Wq_d.rearrange("(co ci) d -> ci co d", ci=128))
            Wk_sb = persist.tile([128, CK, DV], bf16)
            nc.sync.dma_start(Wk_sb, Wk_d.rearrange("(co ci) d -> ci co d", ci=128))
            Wv_sb = persist.tile([128, CK, DV], bf16)
            nc.sync.dma_start(Wv_sb, Wv_d.rearrange("(co ci) d -> ci co d", ci=128))
            Wu_sb = persist.tile([128, C], bf16)
            nc.sync.dma_start(Wu_sb, Wu_d[:, :])
            ones_sb = persist.tile([128, 64], bf16)
            nc.vector.memset(ones_sb, 1.0)
            ebias_sb = persist.tile([128, 1], f32)
            nc.vector.memset(ebias_sb, EXP_BIAS)

            # ---------------- q/k/v projections ----------------
            qT_sb = persist.tile([128, T], bf16)  # partitions 0:64 h0 d, 64:128 h1 d
            kT_sb = persist.tile([128, T], bf16)
            v_sb = persist.tile([128, TJ, DV], bf16)  # [j_inner, j_outer, dv]

            for ts in range(T // 512):
                sl = slice(ts * 512, (ts + 1) * 512)
                q_ps = stage_ps.tile([128, 2, 512], f32, tag="stage")
                for ck in range(CK):
                    nc.tensor.matmul(
                        q_ps[:, 0, :],
                        lhsT=Wq_sb[:, ck, :],
                        rhs=condT_sb[:, ck, sl],
                        start=(ck == 0),
                        stop=(ck == CK - 1),
                    )
                for ck in range(CK):
                    nc.tensor.matmul(
                        q_ps[:, 1, :],
                        lhsT=Wk_sb[:, ck, :],
                        rhs=condT_sb[:, ck, sl],
                        start=(ck == 0),
                        stop=(ck == CK - 1),
                    )
                nc.vector.tensor_copy(qT_sb[:, sl], q_ps[:, 0, :])
                nc.vector.tensor_copy(kT_sb[:, sl], q_ps[:, 1, :])

            for tj in range(TJ):
                v_psum = stage_ps.tile([128, 2, 512], f32, tag="stage")
                for ck in range(CK):
                    nc.tensor.matmul(
                        v_psum[:, 0, 0:DV],
                        lhsT=xT_sb[:, ck, tj * 128 : (tj + 1) * 128],
                        rhs=Wv_sb[:, ck, :],
                        start=(ck == 0),
                        stop=(ck == CK - 1),
                    )
                nc.vector.tensor_copy(v_sb[:, tj, :], v_psum[:, 0, 0:DV])

            # ---------------- attention ----------------
            for ib in range(NIB):
                isl = slice(ib * IB, (ib + 1) * IB)
                pv = pv_ps.tile([128, IB], f32, tag="pv")
                zb = zb_ps.tile([128, IB], f32, tag="zb")
                for tj in range(TJ):
                    jsl = slice(tj * 128, (tj + 1) * 128)
                    st = stage_ps.tile([128, 2, 512], f32, tag="stage")
                    # scores S^T[j, i] per head; K=64 row-packed (h0 rows 0-63,
                    # h1 rows 64-127) -> concurrent on the PE
                    nc.tensor.matmul(
                        st[:, 0, :],
                        lhsT=kT_sb[0:64, jsl],
                        rhs=qT_sb[0:64, isl],
                        start=True,
                        stop=True,
                    )
                    nc.tensor.matmul(
                        st[:, 1, :],
                        lhsT=kT_sb[64:128, jsl],
                        rhs=qT_sb[64:128, isl],
                        start=True,
                        stop=True,
                    )
                    pt = pt_pool.tile([128, 2, 512], bf16)
                    nc.scalar.activation(pt, st, Exp, bias=ebias_sb[:, :], scale=1.0)
                    # PV accumulation, col-packed: h0 -> partitions 0-63,
                    # h1 -> partitions 64-127
                    nc.tensor.matmul(
                        pv[0:64, :],
                        lhsT=v_sb[:, tj, 0:64],
                        rhs=pt[:, 0, :],
                        start=(tj == 0),
                        stop=(tj == TJ - 1),
                        tile_position=(0, 0),
                    )
                    nc.tensor.matmul(
                        pv[64:128, :],
                        lhsT=v_sb[:, tj, 64:128],
                        rhs=pt[:, 1, :],
                        start=(tj == 0),
                        stop=(tj == TJ - 1),
                        tile_position=(0, 64),
                    )
                    # Z broadcast-sum: ones[128,64] -> every output row gets
                    # sum_j P[j,i]
                    nc.tensor.matmul(
                        zb[0:64, :],
                        lhsT=ones_sb,
                        rhs=pt[:, 0, :],
                        start=(tj == 0),
                        stop=(tj == TJ - 1),
                        tile_position=(0, 0),
                    )
                    nc.tensor.matmul(
                        zb[64:128, :],
                        lhsT=ones_sb,
                        rhs=pt[:, 1, :],
                        start=(tj == 0),
                        stop=(tj == TJ - 1),
                        tile_position=(0, 64),
                    )

                zr = work.tile([128, IB], f32, tag="zr")
                nc.vector.reciprocal_approx_fast(zr, zb)
                pvn = work.tile([128, IB], bf16, tag="pvn")
                nc.vector.tensor_mul(pvn, pv, zr)

                # final projection: out[i, :] = sum_dv pvn[dv, i] * Wu[dv, :]
                for isub in range(IB // 128):
                    fo = pv_ps.tile([128, C], f32, tag="pv")
                    nc.tensor.matmul(
                        fo,
                        lhsT=pvn[:, isub * 128 : (isub + 1) * 128],
                        rhs=Wu_sb,
                        start=True,
                        stop=True,
                    )
                    fo_sb = outsb.tile([128, C], f32, tag="fo")
                    nc.vector.tensor_copy(fo_sb, fo)
                    t0 = ib * IB + isub * 128
                    nc.sync.dma_start(out_d[t0 : t0 + 128, :], fo_sb)

    nc.compile()
    return nc


def _get_nc():
    global _BUILT
    if _BUILT is None:
        _BUILT = _build_nc()
    return _BUILT


def kernel(x, condition, W_qk, W_v, W_u, b_u):
    from concourse.bass_utils import run_bass_kernel_spmd

    bf = ml_dtypes.bfloat16
    x = np.asarray(x, dtype=np.float32)
    condition = np.asarray(condition, dtype=np.float32)
    W_qk = np.asarray(W_qk, dtype=np.float32)
    W_v = np.asarray(W_v, dtype=np.float32)
    W_u = np.asarray(W_u, dtype=np.float32)
    b_u = np.asarray(b_u, dtype=np.float32)

    Wq = (W_qk[:, : H * DH] * SCALE).astype(bf)
    Wk = W_qk[:, H * DH :].astype(bf)
    Wv = W_v.astype(bf)
    Wu = W_u.astype(bf)
    condT = np.ascontiguousarray(condition.transpose(0, 2, 1)).astype(bf)
    xT = np.ascontiguousarray(x.transpose(0, 2, 1)).astype(bf)

    in_maps = []
    for core in range(NCORES):
        b = core // 4
        hp = core % 4
        ds = slice(hp * DV, (hp + 1) * DV)
        in_maps.append(
            {
                "condT": condT[b],
                "xT": xT[b],
                "Wq": np.ascontiguousarray(Wq[:, ds]),
                "Wk": np.ascontiguousarray(Wk[:, ds]),
                "Wv": np.ascontiguousarray(Wv[:, ds]),
                "Wu": np.ascontiguousarray(Wu[ds, :]),
            }
        )

    nc = _get_nc()
    res = run_bass_kernel_spmd(nc, in_maps, core_ids=list(range(NCORES)))
    results = res.results

    out = np.zeros((B, T, C), dtype=np.float32)
    for core in range(NCORES):
        out[core // 4] += results[core]["out"]
    out += b_u
    return out


# revision 9
# speedup vs baseline: 1.4302x; 1.4302x over previous
"""CrossAttentionBlock kernel for 8 Trainium2 NeuronCores.

Sharding: 16 (batch, head) pairs -> 8 cores, each core owns one batch b and
two heads (2*hp, 2*hp+1).  Per core:
  qT/kT = (Wq/Wk slice)^T-projection of condition[b]   [128=2*64 d, 4096 t]
  v     = x[b] @ Wv slice                               [4096 j, 128 dv]
  S^T   = kT^T-slices @ qT  (per head, row-packed on the PE)
  P     = exp(S^T - 8)  (ScalarE, PSUM->SBUF, bf16)
  out^T = v^T @ P^T  accumulated over j (col-packed 2 heads), Z via ones-matmul
  final = (out^T / Z)^T @ Wu slice  -> partial [4096, 512] fp32
Host sums the 4 per-batch partials and adds b_u.
"""

import numpy as np
import ml_dtypes

B, T, C = 2, 4096, 512
H, DH = 8, 64
COND = 512
SCALE = (DH // H) ** -0.5  # faithful to reference: 8**-0.5
NCORES = 8
DV = 2 * DH          # per-core head-pair width = 128
CK = COND // 128     # 4 contraction chunks
TJ = T // 128        # 32 key chunks
IB = 512             # query block
NIB = T // IB        # 8
EXP_BIAS = -8.0      # constant shift inside exp; cancels in the softmax ratio

_BUILT = None


def _build_nc():
    import concourse.bass as bass  # noqa: F401
    import concourse.tile as tile
    from concourse import bacc, mybir

    f32 = mybir.dt.float32
    bf16 = mybir.dt.bfloat16

    nc = bacc.Bacc(None)
    condT_d = nc.declare_dram_parameter("condT", [COND, T], bf16, isOutput=False)
    xT_d = nc.declare_dram_parameter("xT", [C, T], bf16, isOutput=False)
    Wq_d = nc.declare_dram_parameter("Wq", [COND, DV], bf16, isOutput=False)
    Wk_d = nc.declare_dram_parameter("Wk", [COND, DV], bf16, isOutput=False)
    Wv_d = nc.declare_dram_parameter("Wv", [C, DV], bf16, isOutput=False)
    Wu_d = nc.declare_dram_parameter("Wu", [DV, C], bf16, isOutput=False)
    out_d = nc.declare_dram_parameter("out", [T, C], f32, isOutput=True)

    Exp = mybir.ActivationFunctionType.Exp

    with tile.TileContext(nc) as tc:
        with (
            tc.tile_pool(name="persist", bufs=1) as persist,
            tc.tile_pool(name="work", bufs=3) as work,
            tc.tile_pool(name="outsb", bufs=3) as outsb,
            tc.tile_pool(name="pt_pool", bufs=4) as pt_pool,
            tc.tile_pool(name="stage_ps", bufs=2, space="PSUM") as stage_ps,
            tc.tile_pool(name="pv_ps", bufs=2, space="PSUM") as pv_ps,
            tc.tile_pool(name="zb_ps", bufs=2, space="PSUM") as zb_ps,
        ):
            # ---------------- load inputs ----------------
            condT_sb = persist.tile([128, CK, T], bf16)
            nc.sync.dma_start(
                condT_sb, condT_d.rearrange("(co ci) t -> ci co t", ci=128)
            )
            xT_sb = persist.tile([128, CK, T], bf16)
            nc.sync.dma_start(xT_sb, xT_d.rearrange("(co ci) t -> ci co t", ci=128))
            Wq_sb = persist.tile([128, CK, DV], bf16)
            nc.sync.dma_start(Wq_sb, Wq_d.rearrange("(co ci) d -> ci co d", ci=128))
            Wk_sb = persist.tile([128, CK, DV], bf16)
            nc.sync.dma_start(Wk_sb, Wk_d.rearrange("(co ci) d -> ci co d", ci=128))
            Wv_sb = persist.tile([128, CK, DV], bf16)
            nc.sync.dma_start(Wv_sb, Wv_d.rearrange("(co ci) d -> ci co d", ci=128))
            Wu_sb = persist.tile([128, C], bf16)
            nc.sync.dma_start(Wu_sb, Wu_d[:, :])
            ones_sb = persist.tile([128, 64], bf16)
            nc.vector.memset(ones_sb, 1.0)
            ebias_sb = persist.tile([128, 1], f32)
            nc.vector.memset(ebias_sb, EXP_BIAS)

            # ---------------- q/k/v projections ----------------
            qT_sb = persist.tile([128, T], bf16)  # partitions 0:64 h0 d, 64:128 h1 d
            kT_sb = persist.tile([128, T], bf16)
            v_sb = persist.tile([128, TJ, DV], bf16)  # [j_inner, j_outer, dv]

            for ts in range(T // 512):
                sl = slice(ts * 512, (ts + 1) * 512)
                q_ps = stage_ps.tile([128, 2, 512], f32, tag="stage")
                for ck in range(CK):
                    nc.tensor.matmul(
                        q_ps[:, 0, :],
                        lhsT=Wq_sb[:, ck, :],
                        rhs=condT_sb[:, ck, sl],
                        start=(ck == 0),
                        stop=(ck == CK - 1),
                    )
                for ck in range(CK):
                    nc.tensor.matmul(
                        q_ps[:, 1, :],
                        lhsT=Wk_sb[:, ck, :],
                        rhs=condT_sb[:, ck, sl],
                        start=(ck == 0),
                        stop=(ck == CK - 1),
                    )
                nc.vector.tensor_copy(qT_sb[:, sl], q_ps[:, 0, :])
                nc.vector.tensor_copy(kT_sb[:, sl], q_ps[:, 1, :])

            for tj in range(TJ):
                v_psum = stage_ps.tile([128, 2, 512], f32, tag="stage")
                for ck in range(CK):
                    nc.tensor.matmul(
                        v_psum[:, 0, 0:DV],
                        lhsT=xT_sb[:, ck, tj * 128 : (tj + 1) * 128],
                        rhs=Wv_sb[:, ck, :],
                        start=(ck == 0),
                        stop=(ck == CK - 1),
                    )
                nc.vector.tensor_copy(v_sb[:, tj, :], v_psum[:, 0, 0:DV])

            # ---------------- attention ----------------
            LAG = 2  # PV/Z trail the scores+exp by LAG j-chunks so the PE
            # never queue-stalls behind the ScalarE exp.
            for ib in range(NIB):
                isl = slice(ib * IB, (ib + 1) * IB)
                pv = pv_ps.tile([128, IB], f32, tag="pv")
                zb = zb_ps.tile([128, IB], f32, tag="zb")
                pts = {}

                def _consume(tj, pts=pts, pv=pv, zb=zb):
                    pt = pts.pop(tj)
                    nc.tensor.matmul(
                        pv[0:64, :],
                        lhsT=v_sb[:, tj, 0:64],
                        rhs=pt[:, 0, :],
                        start=(tj == 0),
                        stop=(tj == TJ - 1),
                        tile_position=(0, 0),
                    )
                    nc.tensor.matmul(
                        pv[64:128, :],
                        lhsT=v_sb[:, tj, 64:128],
                        rhs=pt[:, 1, :],
                        start=(tj == 0),
                        stop=(tj == TJ - 1),
                        tile_position=(0, 64),
                    )
                    nc.tensor.matmul(
                        zb[0:64, :],
                        lhsT=ones_sb,
                        rhs=pt[:, 0, :],
                        start=(tj == 0),
                        stop=(tj == TJ - 1),
                        tile_position=(0, 0),
                    )
                    nc.tensor.matmul(
                        zb[64:128, :],
                        lhsT=ones_sb,
                        rhs=pt[:, 1, :],
                        start=(tj == 0),
                        stop=(tj == TJ - 1),
                        tile_position=(0, 64),
                    )

                for tj in range(TJ):
                    jsl = slice(tj * 128, (tj + 1) * 128)
                    st = stage_ps.tile([128, 2, 512], f32, tag="stage")
                    # scores S^T[j, i] per head; K=64 row-packed (h0 rows 0-63,
                    # h1 rows 64-127) -> concurrent on the PE
                    nc.tensor.matmul(
                        st[:, 0, :],
                        lhsT=kT_sb[0:64, jsl],
                        rhs=qT_sb[0:64, isl],
                        start=True,
                        stop=True,
                    )
                    nc.tensor.matmul(
                        st[:, 1, :],
                        lhsT=kT_sb[64:128, jsl],
                        rhs=qT_sb[64:128, isl],
                        start=True,
                        stop=True,
                    )
                    pt = pt_pool.tile(
                        [128, 2, 512], bf16, name=f"pt_{ib}_{tj}", tag="pt"
                    )
                    nc.scalar.activation(pt, st, Exp, bias=ebias_sb[:, :], scale=1.0)
                    pts[tj] = pt
                    if tj >= LAG:
                        _consume(tj - LAG)
                for tj in range(TJ - LAG, TJ):
                    _consume(tj)

                zr = work.tile([128, IB], f32, tag="zr")
                nc.vector.reciprocal_approx_fast(zr, zb)
                pvn = work.tile([128, IB], bf16, tag="pvn")
                nc.vector.tensor_mul(pvn, pv, zr)

                # final projection: out[i, :] = sum_dv pvn[dv, i] * Wu[dv, :]
                for isub in range(IB // 128):
                    fo = pv_ps.tile([128, C], f32, tag="pv")
                    nc.tensor.matmul(
                        fo,
                        lhsT=pvn[:, isub * 128 : (isub + 1) * 128],
                        rhs=Wu_sb,
                        start=True,
                        stop=True,
                    )
                    fo_sb = outsb.tile([128, C], f32, tag="fo")
                    nc.vector.tensor_copy(fo_sb, fo)
                    t0 = ib * IB + isub * 128
                    nc.sync.dma_start(out_d[t0 : t0 + 128, :], fo_sb)

    nc.compile()
    return nc


def _get_nc():
    global _BUILT
    if _BUILT is None:
        _BUILT = _build_nc()
    return _BUILT


def kernel(x, condition, W_qk, W_v, W_u, b_u):
    from concourse.bass_utils import run_bass_kernel_spmd

    bf = ml_dtypes.bfloat16
    x = np.asarray(x, dtype=np.float32)
    condition = np.asarray(condition, dtype=np.float32)
    W_qk = np.asarray(W_qk, dtype=np.float32)
    W_v = np.asarray(W_v, dtype=np.float32)
    W_u = np.asarray(W_u, dtype=np.float32)
    b_u = np.asarray(b_u, dtype=np.float32)

    Wq = (W_qk[:, : H * DH] * SCALE).astype(bf)
    Wk = W_qk[:, H * DH :].astype(bf)
    Wv = W_v.astype(bf)
    Wu = W_u.astype(bf)
    condT = np.ascontiguousarray(condition.transpose(0, 2, 1)).astype(bf)
    xT = np.ascontiguousarray(x.transpose(0, 2, 1)).astype(bf)

    in_maps = []
    for core in range(NCORES):
        b = core // 4
        hp = core % 4
        ds = slice(hp * DV, (hp + 1) * DV)
        in_maps.append(
            {
                "condT": condT[b],
                "xT": xT[b],
                "Wq": np.ascontiguousarray(Wq[:, ds]),
                "Wk": np.ascontiguousarray(Wk[:, ds]),
                "Wv": np.ascontiguousarray(Wv[:, ds]),
                "Wu": np.ascontiguousarray(Wu[ds, :]),
            }
        )

    nc = _get_nc()
    res = run_bass_kernel_spmd(nc, in_maps, core_ids=list(range(NCORES)))
    results = res.results

    out = np.zeros((B, T, C), dtype=np.float32)
    for core in range(NCORES):
        out[core // 4] += results[core]["out"]
    out += b_u
    return out


# revision 10
# speedup vs baseline: 1.5357x; 1.0737x over previous
"""CrossAttentionBlock kernel for 8 Trainium2 NeuronCores.

Sharding: 16 (batch, head) pairs -> 8 cores, each core owns one batch b and
two heads (2*hp, 2*hp+1).  Per core:
  qT/kT = (Wq/Wk slice)^T-projection of condition[b]   [128=2*64 d, 4096 t]
  v     = x[b] @ Wv slice                               [4096 j, 128 dv]
  S^T   = kT^T-slices @ qT  (per head, row-packed on the PE)
  P     = exp(S^T - 8)  (ScalarE, PSUM->SBUF, bf16)
  out^T = v^T @ P^T  accumulated over j (col-packed 2 heads), Z via ones-matmul
  final = (out^T / Z)^T @ Wu slice  -> partial [4096, 512] fp32
Host sums the 4 per-batch partials and adds b_u.

The attention runs as one flat software-pipelined stream over all
(i-block, j-chunk) pairs: scores+exp for chunk n issue ahead of the PV/Z
consumption of chunk n-LAG, so the in-order PE queue never stalls behind
the ScalarE exp, and ScalarE never gaps at i-block boundaries.
"""

import numpy as np
import ml_dtypes

B, T, C = 2, 4096, 512
H, DH = 8, 64
COND = 512
SCALE = (DH // H) ** -0.5  # faithful to reference: 8**-0.5
NCORES = 8
DV = 2 * DH          # per-core head-pair width = 128
CK = COND // 128     # 4 contraction chunks
TJ = T // 128        # 32 key chunks
IB = 512             # query block
NIB = T // IB        # 8
LAG = 3              # chunks the PV/Z consumption trails the scores/exp
EXP_BIAS = -8.0      # constant shift inside exp; cancels in the softmax ratio

_BUILT = None


def _build_nc():
    import concourse.bass as bass  # noqa: F401
    import concourse.tile as tile
    from concourse import bacc, mybir

    f32 = mybir.dt.float32
    bf16 = mybir.dt.bfloat16

    nc = bacc.Bacc(None)
    condT_d = nc.declare_dram_parameter("condT", [COND, T], bf16, isOutput=False)
    xT_d = nc.declare_dram_parameter("xT", [C, T], bf16, isOutput=False)
    Wq_d = nc.declare_dram_parameter("Wq", [COND, DV], bf16, isOutput=False)
    Wk_d = nc.declare_dram_parameter("Wk", [COND, DV], bf16, isOutput=False)
    Wv_d = nc.declare_dram_parameter("Wv", [C, DV], bf16, isOutput=False)
    Wu_d = nc.declare_dram_parameter("Wu", [DV, C], bf16, isOutput=False)
    out_d = nc.declare_dram_parameter("out", [T, C], f32, isOutput=True)

    Exp = mybir.ActivationFunctionType.Exp

    with tile.TileContext(nc) as tc:
        with (
            tc.tile_pool(name="persist", bufs=1) as persist,
            tc.tile_pool(name="work", bufs=3) as work,
            tc.tile_pool(name="outsb", bufs=3) as outsb,
            tc.tile_pool(name="pt_pool", bufs=LAG + 2) as pt_pool,
            tc.tile_pool(name="stage_ps", bufs=2, space="PSUM") as stage_ps,
            tc.tile_pool(name="pv_ps", bufs=2, space="PSUM") as pv_ps,
            tc.tile_pool(name="zb_ps", bufs=2, space="PSUM") as zb_ps,
        ):
            # ---------------- load inputs (weights first, then sliced) -----
            Wq_sb = persist.tile([128, CK, DV], bf16)
            nc.sync.dma_start(Wq_sb, Wq_d.rearrange("(co ci) d -> ci co d", ci=128))
            Wk_sb = persist.tile([128, CK, DV], bf16)
            nc.sync.dma_start(Wk_sb, Wk_d.rearrange("(co ci) d -> ci co d", ci=128))
            Wv_sb = persist.tile([128, CK, DV], bf16)
            nc.sync.dma_start(Wv_sb, Wv_d.rearrange("(co ci) d -> ci co d", ci=128))
            Wu_sb = persist.tile([128, C], bf16)
            nc.sync.dma_start(Wu_sb, Wu_d[:, :])
            ones_sb = persist.tile([128, 64], bf16)
            nc.vector.memset(ones_sb, 1.0)
            ebias_sb = persist.tile([128, 1], f32)
            nc.vector.memset(ebias_sb, EXP_BIAS)

            condT_r = condT_d.rearrange("(co ci) t -> ci co t", ci=128)
            condT_sb = persist.tile([128, CK, T], bf16)
            for ts in range(T // 512):
                sl = slice(ts * 512, (ts + 1) * 512)
                nc.sync.dma_start(condT_sb[:, :, sl], condT_r[:, :, sl])
            xT_r = xT_d.rearrange("(co ci) t -> ci co t", ci=128)
            xT_sb = persist.tile([128, CK, T], bf16)
            for ts in range(T // 512):
                sl = slice(ts * 512, (ts + 1) * 512)
                nc.sync.dma_start(xT_sb[:, :, sl], xT_r[:, :, sl])

            # ---------------- q/k projections ----------------
            qT_sb = persist.tile([128, T], bf16)  # partitions 0:64 h0 d, 64:128 h1
            kT_sb = persist.tile([128, T], bf16)
            v_sb = persist.tile([128, TJ, DV], bf16)  # [j_inner, j_outer, dv]

            for ts in range(T // 512):
                sl = slice(ts * 512, (ts + 1) * 512)
                q_ps = stage_ps.tile([128, 2, 512], f32, tag="stage")
                for ck in range(CK):
                    nc.tensor.matmul(
                        q_ps[:, 0, :],
                        lhsT=Wq_sb[:, ck, :],
                        rhs=condT_sb[:, ck, sl],
                        start=(ck == 0),
                        stop=(ck == CK - 1),
                    )
                for ck in range(CK):
                    nc.tensor.matmul(
                        q_ps[:, 1, :],
                        lhsT=Wk_sb[:, ck, :],
                        rhs=condT_sb[:, ck, sl],
                        start=(ck == 0),
                        stop=(ck == CK - 1),
                    )
                nc.vector.tensor_copy(qT_sb[:, sl], q_ps[:, 0, :])
                nc.vector.tensor_copy(kT_sb[:, sl], q_ps[:, 1, :])

            # ---------------- flat pipelined attention ----------------
            pvs = {}
            zbs = {}
            pts = {}

            def v_proj_chunk(tj):
                # v[j, dv] for one 128-row j chunk; borrows a zb-pool slot
                v_psum = zb_ps.tile([128, 512], f32, tag="zb", name=f"v_psum_{tj}")
                for ck in range(CK):
                    nc.tensor.matmul(
                        v_psum[:, 0:DV],
                        lhsT=xT_sb[:, ck, tj * 128 : (tj + 1) * 128],
                        rhs=Wv_sb[:, ck, :],
                        start=(ck == 0),
                        stop=(ck == CK - 1),
                    )
                nc.vector.tensor_copy(v_sb[:, tj, :], v_psum[:, 0:DV])

            def finish_block(ib):
                pv = pvs.pop(ib)
                zb = zbs.pop(ib)
                zr = work.tile([128, IB], f32, tag="zr", name=f"zr_{ib}")
                nc.vector.reciprocal_approx_fast(zr, zb)
                pvn = work.tile([128, IB], bf16, tag="pvn", name=f"pvn_{ib}")
                nc.vector.tensor_mul(pvn, pv, zr)
                for isub in range(IB // 128):
                    fo = pv_ps.tile([128, C], f32, tag="pv", name=f"fo_{ib}_{isub}")
                    nc.tensor.matmul(
                        fo,
                        lhsT=pvn[:, isub * 128 : (isub + 1) * 128],
                        rhs=Wu_sb,
                        start=True,
                        stop=True,
                    )
                    fo_sb = outsb.tile([128, C], f32, tag="fo", name=f"fs_{ib}_{isub}")
                    nc.vector.tensor_copy(fo_sb, fo)
                    t0 = ib * IB + isub * 128
                    nc.sync.dma_start(out_d[t0 : t0 + 128, :], fo_sb)

            def consume(n):
                ib, tj = divmod(n, TJ)
                pv = pvs[ib]
                zb = zbs[ib]
                pt = pts.pop(n)
                nc.tensor.matmul(
                    pv[0:64, :],
                    lhsT=v_sb[:, tj, 0:64],
                    rhs=pt[:, 0, :],
                    start=(tj == 0),
                    stop=(tj == TJ - 1),
                    tile_position=(0, 0),
                )
                nc.tensor.matmul(
                    pv[64:128, :],
                    lhsT=v_sb[:, tj, 64:128],
                    rhs=pt[:, 1, :],
                    start=(tj == 0),
                    stop=(tj == TJ - 1),
                    tile_position=(0, 64),
                )
                nc.tensor.matmul(
                    zb[0:64, :],
                    lhsT=ones_sb,
                    rhs=pt[:, 0, :],
                    start=(tj == 0),
                    stop=(tj == TJ - 1),
                    tile_position=(0, 0),
                )
                nc.tensor.matmul(
                    zb[64:128, :],
                    lhsT=ones_sb,
                    rhs=pt[:, 1, :],
                    start=(tj == 0),
                    stop=(tj == TJ - 1),
                    tile_position=(0, 64),
                )
                if tj == TJ - 1:
                    finish_block(ib)

            N = NIB * TJ
            for n in range(N):
                ib, tj = divmod(n, TJ)
                if tj == 0:
                    pvs[ib] = pv_ps.tile(
                        [128, IB], f32, tag="pv", name=f"pv_{ib}"
                    )
                    zbs[ib] = zb_ps.tile(
                        [128, IB], f32, tag="zb", name=f"zb_{ib}"
                    )
                if n < TJ:
                    v_proj_chunk(n)
                isl = slice(ib * IB, (ib + 1) * IB)
                jsl = slice(tj * 128, (tj + 1) * 128)
                st = stage_ps.tile([128, 2, 512], f32, tag="stage", name=f"st_{n}")
                # scores S^T[j, i] per head; K=64 row-packed (h0 rows 0-63,
                # h1 rows 64-127) -> concurrent on the PE
                nc.tensor.matmul(
                    st[:, 0, :],
                    lhsT=kT_sb[0:64, jsl],
                    rhs=qT_sb[0:64, isl],
                    start=True,
                    stop=True,
                )
                nc.tensor.matmul(
                    st[:, 1, :],
                    lhsT=kT_sb[64:128, jsl],
                    rhs=qT_sb[64:128, isl],
                    start=True,
                    stop=True,
                )
                pt = pt_pool.tile([128, 2, 512], bf16, tag="pt", name=f"pt_{n}")
                nc.scalar.activation(pt, st, Exp, bias=ebias_sb[:, :], scale=1.0)
                pts[n] = pt
                if n >= LAG:
                    consume(n - LAG)
            for n in range(N - LAG, N):
                consume(n)

    nc.compile()
    return nc


def _get_nc():
    global _BUILT
    if _BUILT is None:
        _BUILT = _build_nc()
    return _BUILT


def kernel(x, condition, W_qk, W_v, W_u, b_u):
    from concourse.bass_utils import run_bass_kernel_spmd

    bf = ml_dtypes.bfloat16
    x = np.asarray(x, dtype=np.float32)
    condition = np.asarray(condition, dtype=np.float32)
    W_qk = np.asarray(W_qk, dtype=np.float32)
    W_v = np.asarray(W_v, dtype=np.float32)
    W_u = np.asarray(W_u, dtype=np.float32)
    b_u = np.asarray(b_u, dtype=np.float32)

    Wq = (W_qk[:, : H * DH] * SCALE).astype(bf)
    Wk = W_qk[:, H * DH :].astype(bf)
    Wv = W_v.astype(bf)
    Wu = W_u.astype(bf)
    condT = np.ascontiguousarray(condition.transpose(0, 2, 1)).astype(bf)
    xT = np.ascontiguousarray(x.transpose(0, 2, 1)).astype(bf)

    in_maps = []
    for core in range(NCORES):
        b = core // 4
        hp = core % 4
        ds = slice(hp * DV, (hp + 1) * DV)
        in_maps.append(
            {
                "condT": condT[b],
                "xT": xT[b],
                "Wq": np.ascontiguousarray(Wq[:, ds]),
                "Wk": np.ascontiguousarray(Wk[:, ds]),
                "Wv": np.ascontiguousarray(Wv[:, ds]),
                "Wu": np.ascontiguousarray(Wu[ds, :]),
            }
        )

    nc = _get_nc()
    res = run_bass_kernel_spmd(nc, in_maps, core_ids=list(range(NCORES)))
    results = res.results

    out = np.zeros((B, T, C), dtype=np.float32)
    for core in range(NCORES):
        out[core // 4] += results[core]["out"]
    out += b_u
    return out
